# revision 19
# baseline (speedup 1.0000x reference)
"""Trainium2 Bass kernel for nn_AssocKernelAdapter (sparse_attention).

Strategy (8 NeuronCores, SPMD, no collectives):
  - Each core redundantly computes the prep (RMSNorm + qk/v projections +
    L2-normalize) for the full sequence in TRANSPOSED space (feature-major),
    then computes attention scores / softmax / attn@v / out-proj for its own
    T/8 = 256 query rows only (all 12 heads).
  - All cores run the same instruction graph; per-core behavior comes from a
    host-side cyclic rotation of the inputs: core r receives x^T (and the
    bias constants) rolled by -r*TL along the sequence axis so that its own
    query rows always occupy columns [0, TL). The host un-rotates the attn
    output when assembling.
  - Score math: softmax(clip(-dist/sig^2) + log(gate) + emo) is rewritten as
      softmax_j( c*sim[i,j] + log(gate[i,j]) + s1*e[i]*K[i,j] + const(i) )
    with c = 2/sig^2, K[i,j] = exp(-|i-j|/lam), s1 = GAM/std.  Row-constant
    terms cancel in softmax, so on device:
      p = exp(c*sim) * EB,   EB = max(gate,1e-4) * exp(s1*e_i*K)   (per core,
    shared across heads), then normalize by the row sum.
  - bf16 matmuls/storage, f32 reductions.  attn is written to HBM in bf16 and
    cast to f32 on host.
"""

import os
import sys

import numpy as np

sys.path.insert(0, "/opt/trn_rl_repo")

import concourse.bass as bass
import concourse.mybir as mybir
from concourse import bacc
import concourse.tile as tile
from concourse.bass_utils import run_bass_kernel_spmd

try:
    import ml_dtypes

    BF16 = ml_dtypes.bfloat16
except ImportError:  # pragma: no cover
    BF16 = np.float32

E = 768
H = 12
D = 64
WIN = 128
STRIDE = 8
LAM = 8.0
GAM = 0.5
SIGMA_MIN = 1e-4
NCORES = 8

F32 = mybir.dt.float32
BF = mybir.dt.bfloat16
AF = mybir.ActivationFunctionType
OP = mybir.AluOpType

LAST_EXEC_NS = None
LAST_TRACE_DIR = None


def _install_ntff_hook():
    """Provide antenv.axon_hooks (missing in this container) via ctypes."""
    import contextlib
    import ctypes
    import types

    if "antenv.axon_hooks" in sys.modules:
        return
    so_path = "/opt/axon/libaxon_pjrt.so"
    try:
        lib = ctypes.CDLL(so_path)
        if not hasattr(lib, "axon_start_nrt_profile"):
            return
    except OSError:
        return
    lib.axon_start_nrt_profile.argtypes = [
        ctypes.POINTER(ctypes.c_int64),
        ctypes.c_size_t,
    ]
    lib.axon_start_nrt_profile.restype = ctypes.c_int64
    lib.axon_stop_nrt_profile.argtypes = [ctypes.c_char_p]
    lib.axon_stop_nrt_profile.restype = ctypes.c_int64

    @contextlib.contextmanager
    def _hook(output_dir, device_ids):
        import jax

        jax.devices()
        if device_ids:
            ids = (ctypes.c_int64 * len(device_ids))(*device_ids)
            rc = lib.axon_start_nrt_profile(ids, len(device_ids))
        else:
            rc = lib.axon_start_nrt_profile(None, 0)
        if rc != 0:
            raise RuntimeError(f"axon_start_nrt_profile rc={rc}")
        try:
            yield
        finally:
            n = lib.axon_stop_nrt_profile(str(output_dir).encode())
            print(f"ntff profile: {n} file(s) written to {output_dir}",
                  file=sys.stderr)

    mod = types.ModuleType("antenv.axon_hooks")
    mod.get_axon_ntff_profile_hook = lambda: _hook
    mod.set_axon_ntff_profile_hook = lambda h: None
    sys.modules["antenv.axon_hooks"] = mod


def _bcast_part(ap, n):
    """View a [1, ...] SBUF/DRAM AP broadcast to n partitions (step-0)."""
    return bass.AP(
        tensor=ap.tensor,
        offset=ap.offset,
        ap=[[0, n]] + [list(x) for x in ap.ap[1:]],
    )


def build_bass(T, c_scale, need_clip=False):
    """Build the per-core Bass graph (identical on all cores)."""
    TL = T // NCORES          # local query rows per core
    ICN = max(TL // 128, 1)   # i-chunks of 128 local rows
    assert TL % 128 == 0
    EC = E // 128             # feature chunks (6)
    NW = min(512, T)          # matmul N chunk (PSUM bank = 512 f32)
    JW = T // NW              # j chunks per row tile
    JB = T // 128             # 128-wide j blocks (for transpose / attn@v)
    TC = T // 128             # full-sequence row chunks (for v)

    nc = bacc.Bacc(trn_type="TRN2")

    # ---- DRAM parameters (host supplies exact layouts) ----
    xT = nc.declare_dram_parameter("xT", [E, T], BF, False)       # rotated
    wqkT = nc.declare_dram_parameter("wqkT", [E, E], BF, False)   # [k, e], rms folded
    wvT = nc.declare_dram_parameter("wvT", [E, E], BF, False)
    woutT = nc.declare_dram_parameter("woutT", [E, E], BF, False)
    expG = nc.declare_dram_parameter("expG", [TL, T], BF, False)  # local rows, rotated
    Kmat = nc.declare_dram_parameter("Kmat", [TL, T], F32, False)
    eloc = nc.declare_dram_parameter("eloc", [TL, 1], F32, False)
    efull = nc.declare_dram_parameter("efull", [128, T // 128], F32, False)
    rowK = nc.declare_dram_parameter("rowK", [128, T // 128], F32, False)
    rowK2 = nc.declare_dram_parameter("rowK2", [128, T // 128], F32, False)
    ident = nc.declare_dram_parameter("ident", [128, 128], BF, False)
    ones1 = nc.declare_dram_parameter("ones1", [128, 1], BF, False)
    bd = nc.declare_dram_parameter("bd", [128, EC, H], BF, False)  # blockdiag sel

    attn_o = nc.declare_dram_parameter("attn", [H, TL, T], BF, True)
    out_o = nc.declare_dram_parameter("out", [TL, E], F32, True)

    from contextlib import ExitStack

    with ExitStack() as top:
        tc = top.enter_context(tile.TileContext(nc))

        consts = top.enter_context(tc.tile_pool(name="consts", bufs=1))
        persist = top.enter_context(tc.tile_pool(name="persist", bufs=1))
        dscr = top.enter_context(tc.tile_pool(name="dscr", bufs=1, space="DRAM"))

        id_sb = consts.tile([128, 128], BF)
        nc.sync.dma_start(out=id_sb, in_=ident[:])
        ones_sb = consts.tile([128, 1], BF)
        nc.sync.dma_start(out=ones_sb, in_=ones1[:])
        bd_sb = consts.tile([128, EC, H], BF)
        nc.sync.dma_start(out=bd_sb, in_=bd[:])

        wout_sb = consts.tile([128, EC, E], BF)
        for kc in range(EC):
            nc.sync.dma_start(
                out=wout_sb[:, kc, :], in_=woutT[kc * 128:(kc + 1) * 128, :]
            )

        # persistent products of the prep phase
        qhT_sb = persist.tile([128, EC, T], BF)      # normalized q^T (24KB/p)
        v_sb = persist.tile([128, TC, E], BF)        # v, natural layout
        eb_sb = persist.tile([128, ICN, T], BF)      # exp-bias factor (local rows)

        # ================= PREP =================
        with ExitStack() as prep:
            wpool = prep.enter_context(tc.tile_pool(name="wpool", bufs=1))
            wqk_sb = wpool.tile([128, EC, E], BF)
            wv_sb = wpool.tile([128, EC, E], BF)
            for kc in range(EC):
                nc.sync.dma_start(
                    out=wqk_sb[:, kc, :], in_=wqkT[kc * 128:(kc + 1) * 128, :]
                )
                nc.sync.dma_start(
                    out=wv_sb[:, kc, :], in_=wvT[kc * 128:(kc + 1) * 128, :]
                )

            bigH = prep.enter_context(tc.tile_pool(name="bigH", bufs=1))
            hT_sb = bigH.tile([128, EC, T], BF)

            # ---- phase 1: RMS norm (x^T -> h^T) ----
            with ExitStack() as s1ctx:
                bigX = s1ctx.enter_context(tc.tile_pool(name="bigX", bufs=1))
                mpool = s1ctx.enter_context(
                    tc.tile_pool(name="msum_ps", bufs=1, space="PSUM")
                )
                spool = s1ctx.enter_context(tc.tile_pool(name="rs_small", bufs=1))
                xT_sb = bigX.tile([128, EC, T], BF)
                sqx_sb = bigX.tile([128, EC, T], BF)
                for ec in range(EC):
                    nc.sync.dma_start(
                        out=xT_sb[:, ec, :], in_=xT[ec * 128:(ec + 1) * 128, :]
                    )
                    nc.scalar.activation(
                        sqx_sb[:, ec, :], xT_sb[:, ec, :], AF.Square
                    )
                msum_ps = mpool.tile([1, T], F32)
                for wc in range(JW):
                    for ec in range(EC):
                        nc.tensor.matmul(
                            msum_ps[:, wc * NW:(wc + 1) * NW],
                            ones_sb,
                            sqx_sb[:, ec, wc * NW:(wc + 1) * NW],
                            start=(ec == 0),
                            stop=(ec == EC - 1),
                        )
                # rs_x = 1/sqrt(mean + 1e-6)
                eps1 = spool.tile([1, 1], F32)
                nc.vector.memset(eps1, 1e-6)
                srt = spool.tile([1, T], F32)
                for wc in range(JW):
                    sl = slice(wc * NW, (wc + 1) * NW)
                    nc.scalar.activation(
                        srt[:, sl], msum_ps[:, sl], AF.Sqrt,
                        bias=eps1[:], scale=1.0 / E
                    )
                rsx = spool.tile([1, T], F32)
                nc.vector.reciprocal_approx_fast(rsx, srt)
                rsxb = spool.tile([1, T], BF)
                nc.vector.tensor_copy(rsxb, rsx)
                rsx_d = dscr.tile([1, T], BF)
                nc.sync.dma_start(out=rsx_d, in_=rsxb)
                rsx_e = spool.tile([128, T], BF)
                nc.sync.dma_start(out=rsx_e, in_=_bcast_part(rsx_d[:], 128))
                for ec in range(EC):
                    nc.vector.tensor_mul(
                        hT_sb[:, ec, :], xT_sb[:, ec, :], rsx_e
                    )

            # ---- phase 2: projections + q normalize + v ----
            with ExitStack() as s2ctx:
                mm_ps = s2ctx.enter_context(
                    tc.tile_pool(name="mm_ps", bufs=2, space="PSUM")
                )
                nrm_ps = s2ctx.enter_context(
                    tc.tile_pool(name="nrm_ps", bufs=1, space="PSUM")
                )
                tr = s2ctx.enter_context(tc.tile_pool(name="prep_tr", bufs=2))

                for ec in range(EC):
                    shT = tr.tile([128, T], BF, tag="shT")
                    sqs = tr.tile([128, T], BF, tag="sqs")
                    nrm2 = nrm_ps.tile([2, T], F32, tag="nrm")
                    for wc in range(JW):
                        ps = mm_ps.tile([128, NW], F32, tag="mm")
                        for kc in range(EC):
                            nc.tensor.matmul(
                                ps,
                                wqk_sb[:, kc, ec * 128:(ec + 1) * 128],
                                hT_sb[:, kc, wc * NW:(wc + 1) * NW],
                                start=(kc == 0),
                                stop=(kc == EC - 1),
                            )
                        nc.vector.tensor_copy(shT[:, wc * NW:(wc + 1) * NW], ps)
                        nc.scalar.activation(
                            sqs[:, wc * NW:(wc + 1) * NW], ps, AF.Square
                        )
                        nc.tensor.matmul(
                            nrm2[:, wc * NW:(wc + 1) * NW],
                            bd_sb[:, ec, 2 * ec:2 * ec + 2],
                            sqs[:, wc * NW:(wc + 1) * NW],
                            start=True,
                            stop=True,
                        )
                    # rsq = 1/sqrt(nrm2)  [2, T]
                    srq = tr.tile([2, T], F32, tag="srq")
                    for wc in range(JW):
                        sl = slice(wc * NW, (wc + 1) * NW)
                        nc.scalar.activation(srq[:, sl], nrm2[:, sl], AF.Sqrt)
                    rsq = tr.tile([2, T], F32, tag="rsq")
                    nc.vector.reciprocal_approx_fast(rsq, srq)
                    rsqb = tr.tile([2, T], BF, tag="rsqb")
                    nc.vector.tensor_copy(rsqb, rsq)
                    rsq_d = dscr.tile([2, T], BF, tag="rsq_d")
                    nc.sync.dma_start(out=rsq_d, in_=rsqb)
                    rsq_e = tr.tile([128, T], BF, tag="rsq_e")
                    nc.sync.dma_start(
                        out=rsq_e[0:64, :], in_=_bcast_part(rsq_d[0:1, :], 64)
                    )
                    nc.sync.dma_start(
                        out=rsq_e[64:128, :], in_=_bcast_part(rsq_d[1:2, :], 64)
                    )
                    nc.vector.tensor_mul(qhT_sb[:, ec, :], shT, rsq_e)

                # ---- v projection (natural layout) ----
                for tcl in range(TC):
                    ps = mm_ps.tile([128, E], F32, tag="mm")
                    for n0 in range(0, E, 512):
                        n1 = min(n0 + 512, E)
                        for kc in range(EC):
                            nc.tensor.matmul(
                                ps[:, n0:n1],
                                hT_sb[:, kc, tcl * 128:(tcl + 1) * 128],
                                wv_sb[:, kc, n0:n1],
                                start=(kc == 0),
                                stop=(kc == EC - 1),
                            )
                    for n0 in range(0, E, 512):
                        n1 = min(n0 + 512, E)
                        nc.scalar.copy(v_sb[:, tcl, n0:n1], ps[:, n0:n1])

            # ---- phase 3: emotion bias stats + exp-bias factor ----
            with ExitStack() as s3ctx:
                sm = s3ctx.enter_context(tc.tile_pool(name="emo_small", bufs=1))
                ebtr = s3ctx.enter_context(tc.tile_pool(name="eb_tr", bufs=2))
                TC128 = T // 128
                e2d = sm.tile([128, TC128], F32)
                rk = sm.tile([128, TC128], F32)
                rk2 = sm.tile([128, TC128], F32)
                nc.sync.dma_start(out=e2d, in_=efull[:])
                nc.sync.dma_start(out=rk, in_=rowK[:])
                nc.sync.dma_start(out=rk2, in_=rowK2[:])
                t1 = sm.tile([128, TC128], F32)
                nc.vector.tensor_mul(t1, e2d, rk)
                s_eK = sm.tile([1, 1], F32)
                nc.gpsimd.tensor_reduce(
                    s_eK, t1, axis=mybir.AxisListType.XYZWC, op=OP.add
                )
                ee = sm.tile([128, TC128], F32)
                nc.vector.tensor_mul(ee, e2d, e2d)
                t2 = sm.tile([128, TC128], F32)
                nc.vector.tensor_mul(t2, ee, rk2)
                s_e2K2 = sm.tile([1, 1], F32)
                nc.gpsimd.tensor_reduce(
                    s_e2K2, t2, axis=mybir.AxisListType.XYZWC, op=OP.add
                )
                # var = (s_e2K2 - s_eK^2/T^2) / (T^2-1); std = max(sqrt,1e-6)
                u = sm.tile([1, 1], F32)
                nc.vector.tensor_mul(u, s_eK, s_eK)
                var = sm.tile([1, 1], F32)
                nc.vector.scalar_tensor_tensor(
                    var, u, -1.0 / (T * T), s_e2K2, OP.mult, OP.add
                )
                std = sm.tile([1, 1], F32)
                nc.scalar.activation(std, var, AF.Sqrt, scale=1.0 / (T * T - 1))
                nc.vector.tensor_scalar_max(std, std, 1e-6)
                istd = sm.tile([1, 1], F32)
                nc.vector.reciprocal(istd, std)
                s1 = sm.tile([1, 1], F32)
                nc.vector.tensor_scalar_mul(s1, istd, GAM)
                s1_d = dscr.tile([1, 1], F32)
                nc.sync.dma_start(out=s1_d, in_=s1)
                s1b = sm.tile([128, 1], F32)
                nc.sync.dma_start(out=s1b, in_=_bcast_part(s1_d[:], 128))

                # ---- exp-bias factor per local row chunk ----
                elc = sm.tile([128, ICN], F32)
                nc.sync.dma_start(
                    out=elc,
                    in_=eloc[:].rearrange("(c p) one -> p (c one)", p=128),
                )
                for icn in range(ICN):
                    kt = ebtr.tile([128, T], F32, tag="kt")
                    nc.sync.dma_start(
                        out=kt, in_=Kmat[icn * 128:(icn + 1) * 128, :]
                    )
                    s1e = sm.tile([128, 1], F32, tag="s1e")
                    nc.vector.tensor_mul(s1e, elc[:, icn:icn + 1], s1b)
                    eb1 = ebtr.tile([128, T], BF, tag="eb1")
                    nc.scalar.activation(eb1, kt, AF.Exp, scale=s1e)
                    gt = ebtr.tile([128, T], BF, tag="gt")
                    nc.sync.dma_start(
                        out=gt, in_=expG[icn * 128:(icn + 1) * 128, :]
                    )
                    nc.vector.tensor_mul(eb_sb[:, icn, :], eb1, gt)

        # ================= HEAD LOOP =================
        NW2 = min(1024, T)
        JW2 = T // NW2
        with ExitStack() as hl:
            ps_s = hl.enter_context(tc.tile_pool(name="ps_s", bufs=3, space="PSUM"))
            ps_av = hl.enter_context(
                tc.tile_pool(name="ps_av", bufs=2, space="PSUM")
            )
            hp = hl.enter_context(tc.tile_pool(name="hp", bufs=3))
            pTp = hl.enter_context(tc.tile_pool(name="pTp", bufs=2))
            mrg = hl.enter_context(tc.tile_pool(name="mrg", bufs=1))

            mergedT = mrg.tile([128, EC, TL], BF)

            for h in range(H):
                ec, half = h // 2, h % 2
                r0 = half * 64
                pT = pTp.tile([128, JB, TL], BF, tag="pT")
                for icn in range(ICN):
                    p_t = hp.tile([128, T], BF, tag="p")
                    for wc in range(JW2):
                        ps = ps_s.tile([128, NW2], F32, tag="s")
                        for sub in range(NW2 // NW):
                            o = sub * NW
                            j0 = wc * NW2 + o
                            nc.tensor.matmul(
                                ps[:, o:o + NW],
                                qhT_sb[r0:r0 + 64, ec,
                                       icn * 128:(icn + 1) * 128],
                                qhT_sb[r0:r0 + 64, ec, j0:j0 + NW],
                                start=True,
                                stop=True,
                            )
                        sl = slice(wc * NW2, (wc + 1) * NW2)
                        if need_clip:
                            cl = hp.tile([128, NW2], F32, tag="cl")
                            nc.vector.tensor_scalar(
                                cl, ps, c_scale, -c_scale, OP.mult, OP.add
                            )
                            nc.vector.tensor_scalar(
                                cl, cl, 12.0, -12.0, OP.min, OP.max
                            )
                            nc.scalar.activation(p_t[:, sl], cl, AF.Exp)
                        else:
                            nc.scalar.activation(
                                p_t[:, sl], ps, AF.Exp, scale=c_scale
                            )
                    # p *= EB; row sums via 4x-mode tensor_scalar accumulate
                    nc.vector.tensor_mul(p_t, p_t, eb_sb[:, icn, :])
                    rtot = hp.tile([128, 1], F32, tag="rtot")
                    nc.vector.tensor_scalar(
                        p_t, p_t, 1.0, 0.0, OP.mult, OP.add, accum_out=rtot
                    )
                    recip = hp.tile([128, 1], F32, tag="recip")
                    nc.vector.reciprocal_approx_fast(recip, rtot)
                    pn = hp.tile([128, T], BF, tag="pn")
                    nc.vector.tensor_scalar_mul(pn, p_t, recip)
                    nc.sync.dma_start(
                        out=attn_o[h, icn * 128:(icn + 1) * 128, :], in_=pn
                    )
                    # transpose pn into pT (DMA xbar transpose)
                    nc.sync.dma_start_transpose(
                        pT[:, :, icn * 128:(icn + 1) * 128], pn
                    )
                # attn @ v (transposed output: [64, TL])
                av = ps_av.tile([64, TL], F32, tag="av")
                for jc in range(JB):
                    nc.tensor.matmul(
                        av,
                        v_sb[:, jc, h * 64:(h + 1) * 64],
                        pT[:, jc, :],
                        start=(jc == 0),
                        stop=(jc == JB - 1),
                    )
                nc.scalar.copy(mergedT[r0:r0 + 64, ec, :], av)

            # ---- out projection ----
            op_sb = hl.enter_context(tc.tile_pool(name="op_sb", bufs=2))
            for icn in range(ICN):
                ps = ps_s.tile([128, E], F32, tag="s")
                for n0 in range(0, E, 512):
                    n1 = min(n0 + 512, E)
                    for kc in range(EC):
                        nc.tensor.matmul(
                            ps[:, n0:n1],
                            mergedT[:, kc, icn * 128:(icn + 1) * 128],
                            wout_sb[:, kc, n0:n1],
                            start=(kc == 0),
                            stop=(kc == EC - 1),
                        )
                ot = op_sb.tile([128, E], F32, tag="ot")
                for n0 in range(0, E, 512):
                    n1 = min(n0 + 512, E)
                    nc.vector.tensor_copy(ot[:, n0:n1], ps[:, n0:n1])
                nc.sync.dma_start(
                    out=out_o[icn * 128:(icn + 1) * 128, :], in_=ot
                )

    nc.finalize()
    return nc


def _sigmoid(z):
    return 1.0 / (1.0 + np.exp(-z))


def make_host_inputs(x, emotion, w_qk, w_v, w_out, rms_weight, sigma, beta_local,
                     beta_global, T):
    """Build the per-core in_maps (list of dicts) plus compile-time scalars."""
    TL = T // NCORES
    x2 = np.asarray(x, np.float32).reshape(T, E)
    e = np.asarray(emotion, np.float32).reshape(T)

    sig = max(float(sigma), SIGMA_MIN)
    c = 2.0 / (sig * sig)
    need_clip = (4.0 / (sig * sig)) > 12.0

    rmsw = np.asarray(rms_weight, np.float32).reshape(E)
    wqkT = (np.asarray(w_qk, np.float32) * rmsw[None, :]).T.astype(BF16)
    wvT = (np.asarray(w_v, np.float32) * rmsw[None, :]).T.astype(BF16)
    woutT = np.asarray(w_out, np.float32).T.astype(BF16)
    wqkT = np.ascontiguousarray(wqkT)
    wvT = np.ascontiguousarray(wvT)
    woutT = np.ascontiguousarray(woutT)

    idx = np.arange(T)
    adiff = np.abs(idx[None, :] - idx[:, None]).astype(np.float32)
    local = (adiff <= WIN).astype(np.float32)
    strided = np.broadcast_to(
        ((idx % STRIDE) == 0).astype(np.float32)[None, :], (T, T)
    )
    gate = _sigmoid(beta_local) * local + _sigmoid(beta_global) * strided
    expG_full = np.maximum(gate, 1e-4).astype(np.float32)
    Kfull = np.exp(-adiff / max(LAM, 1e-6)).astype(np.float32)
    rowK = Kfull.sum(1).astype(np.float32)
    rowK2 = (Kfull * Kfull).sum(1).astype(np.float32)

    xT_full = np.ascontiguousarray(x2.T).astype(BF16)  # [E, T]
    ident = np.eye(128, dtype=np.float32).astype(BF16)
    ones1 = np.ones((128, 1), np.float32).astype(BF16)
    EC = E // 128
    bd = np.zeros((128, EC, H), np.float32)
    for ec in range(EC):
        bd[0:64, ec, 2 * ec] = 1.0
        bd[64:128, ec, 2 * ec + 1] = 1.0
    bd = bd.astype(BF16)

    e2d = np.ascontiguousarray(e.reshape(128, T // 128))
    rowK2d = np.ascontiguousarray(rowK.reshape(128, T // 128))
    rowK22d = np.ascontiguousarray(rowK2.reshape(128, T // 128))

    in_maps = []
    for r in range(NCORES):
        rows = slice(r * TL, (r + 1) * TL)
        in_maps.append(
            dict(
                xT=np.ascontiguousarray(np.roll(xT_full, -r * TL, axis=1)),
                wqkT=wqkT,
                wvT=wvT,
                woutT=woutT,
                expG=np.ascontiguousarray(
                    np.roll(expG_full[rows], -r * TL, axis=1)
                ).astype(BF16),
                Kmat=np.ascontiguousarray(np.roll(Kfull[rows], -r * TL, axis=1)),
                eloc=np.ascontiguousarray(e[rows].reshape(TL, 1)),
                efull=e2d,
                rowK=rowK2d,
                rowK2=rowK22d,
                ident=ident,
                ones1=ones1,
                bd=bd,
            )
        )
    return in_maps, c, need_clip


def assemble_outputs(results, T):
    TL = T // NCORES
    out = np.zeros((1, T, E), np.float32)
    attn = np.zeros((1, H, T, T), np.float32)
    for r in range(NCORES):
        res = results[r]
        out[0, r * TL:(r + 1) * TL] = np.asarray(res["out"], np.float32)
        a = np.asarray(res["attn"], np.float32)  # [H, TL, T] rotated
        attn[0, :, r * TL:(r + 1) * TL, :] = np.roll(a, r * TL, axis=2)
    return out, attn


def kernel(x, emotion_activation, w_qk, w_v, w_out, rms_weight, sigma,
           beta_local, beta_global):
    global LAST_EXEC_NS
    x = np.asarray(x)
    B, T, _ = x.shape
    assert B == 1

    in_maps, c, need_clip = make_host_inputs(
        x, emotion_activation, w_qk, w_v, w_out, rms_weight, sigma,
        beta_local, beta_global, T
    )
    nc = build_bass(T, c, need_clip)

    trace = os.environ.get("KERNEL_TRACE", "0") == "1"
    if trace:
        _install_ntff_hook()
        global LAST_TRACE_DIR
        import tempfile

        LAST_TRACE_DIR = tempfile.mkdtemp(prefix="kernel_trace_")
        res = run_bass_kernel_spmd(
            nc, in_maps, core_ids=list(range(NCORES)), trace=True,
            tmpdir=LAST_TRACE_DIR,
        )
    else:
        res = run_bass_kernel_spmd(
            nc, in_maps, core_ids=list(range(NCORES)), trace=False
        )
    LAST_EXEC_NS = res.exec_time_ns
    out, attn = assemble_outputs(res.results, T)
    return out, attn


# revision 22
# speedup vs baseline: 1.3286x; 1.3286x over previous
"""Trainium2 Bass kernel for nn_AssocKernelAdapter (sparse_attention).

Strategy (8 NeuronCores, SPMD, no collectives):
  - Each core redundantly computes the prep (RMSNorm + qk/v projections +
    L2-normalize) for the full sequence in TRANSPOSED space (feature-major),
    then computes attention scores / softmax / attn@v / out-proj for its own
    T/8 = 256 query rows only (all 12 heads).
  - All cores run the same instruction graph; per-core behavior comes from a
    host-side cyclic rotation of the inputs: core r receives x^T (and the
    bias constants) rolled by -r*TL along the sequence axis so that its own
    query rows always occupy columns [0, TL). The host un-rotates the attn
    output when assembling.
  - Score math: softmax(clip(-dist/sig^2) + log(gate) + emo) is rewritten as
      softmax_j( c*sim[i,j] + log(gate[i,j]) + s1*e[i]*K[i,j] + const(i) )
    with c = 2/sig^2, K[i,j] = exp(-|i-j|/lam), s1 = GAM/std.  Row-constant
    terms cancel in softmax, so on device:
      p = exp(c*sim) * EB,   EB = max(gate,1e-4) * exp(s1*e_i*K)   (per core,
    shared across heads), then normalize by the row sum.
  - bf16 matmuls/storage, f32 reductions.  attn is written to HBM in bf16 and
    cast to f32 on host.
"""

import os
import sys

import numpy as np

sys.path.insert(0, "/opt/trn_rl_repo")

import concourse.bass as bass
import concourse.mybir as mybir
from concourse import bacc
import concourse.tile as tile
from concourse.bass_utils import run_bass_kernel_spmd

try:
    import ml_dtypes

    BF16 = ml_dtypes.bfloat16
except ImportError:  # pragma: no cover
    BF16 = np.float32

E = 768
H = 12
D = 64
WIN = 128
STRIDE = 8
LAM = 8.0
GAM = 0.5
SIGMA_MIN = 1e-4
NCORES = 8

F32 = mybir.dt.float32
BF = mybir.dt.bfloat16
AF = mybir.ActivationFunctionType
OP = mybir.AluOpType

LAST_EXEC_NS = None
LAST_TRACE_DIR = None


def _install_ntff_hook():
    """Provide antenv.axon_hooks (missing in this container) via ctypes."""
    import contextlib
    import ctypes
    import types

    if "antenv.axon_hooks" in sys.modules:
        return
    so_path = "/opt/axon/libaxon_pjrt.so"
    try:
        lib = ctypes.CDLL(so_path)
        if not hasattr(lib, "axon_start_nrt_profile"):
            return
    except OSError:
        return
    lib.axon_start_nrt_profile.argtypes = [
        ctypes.POINTER(ctypes.c_int64),
        ctypes.c_size_t,
    ]
    lib.axon_start_nrt_profile.restype = ctypes.c_int64
    lib.axon_stop_nrt_profile.argtypes = [ctypes.c_char_p]
    lib.axon_stop_nrt_profile.restype = ctypes.c_int64

    @contextlib.contextmanager
    def _hook(output_dir, device_ids):
        import jax

        jax.devices()
        if device_ids:
            ids = (ctypes.c_int64 * len(device_ids))(*device_ids)
            rc = lib.axon_start_nrt_profile(ids, len(device_ids))
        else:
            rc = lib.axon_start_nrt_profile(None, 0)
        if rc != 0:
            raise RuntimeError(f"axon_start_nrt_profile rc={rc}")
        try:
            yield
        finally:
            n = lib.axon_stop_nrt_profile(str(output_dir).encode())
            print(f"ntff profile: {n} file(s) written to {output_dir}",
                  file=sys.stderr)

    mod = types.ModuleType("antenv.axon_hooks")
    mod.get_axon_ntff_profile_hook = lambda: _hook
    mod.set_axon_ntff_profile_hook = lambda h: None
    sys.modules["antenv.axon_hooks"] = mod


def _bcast_part(ap, n):
    """View a [1, ...] SBUF/DRAM AP broadcast to n partitions (step-0)."""
    return bass.AP(
        tensor=ap.tensor,
        offset=ap.offset,
        ap=[[0, n]] + [list(x) for x in ap.ap[1:]],
    )


def build_bass(T, c_scale, need_clip=False):
    """Build the per-core Bass graph (identical on all cores)."""
    TL = T // NCORES          # local query rows per core
    ICN = max(TL // 128, 1)   # i-chunks of 128 local rows
    assert TL % 128 == 0
    EC = E // 128             # feature chunks (6)
    NW = min(512, T)          # matmul N chunk (PSUM bank = 512 f32)
    JW = T // NW              # j chunks per row tile
    JB = T // 128             # 128-wide j blocks (for transpose / attn@v)
    TC = T // 128             # full-sequence row chunks (for v)

    nc = bacc.Bacc(trn_type="TRN2")

    # ---- DRAM parameters (host supplies exact layouts) ----
    xT = nc.declare_dram_parameter("xT", [E, T], BF, False)       # rotated
    wqkT = nc.declare_dram_parameter("wqkT", [E, E], BF, False)   # [k, e], rms folded
    wvT = nc.declare_dram_parameter("wvT", [E, E], BF, False)
    woutT = nc.declare_dram_parameter("woutT", [E, E], BF, False)
    expG = nc.declare_dram_parameter("expG", [TL, T], BF, False)  # local rows, rotated
    logGc = nc.declare_dram_parameter("logGc", [TL, T], F32, False)  # log(gate)/c
    Kmat = nc.declare_dram_parameter("Kmat", [TL, T], F32, False)
    eloc = nc.declare_dram_parameter("eloc", [TL, 1], F32, False)
    efull = nc.declare_dram_parameter("efull", [128, T // 128], F32, False)
    rowK = nc.declare_dram_parameter("rowK", [128, T // 128], F32, False)
    rowK2 = nc.declare_dram_parameter("rowK2", [128, T // 128], F32, False)
    ident = nc.declare_dram_parameter("ident", [128, 128], BF, False)
    identf = nc.declare_dram_parameter(
        "identf", [128, 128], mybir.dt.float32r, False
    )
    ones1 = nc.declare_dram_parameter("ones1", [128, 1], BF, False)
    bd = nc.declare_dram_parameter("bd", [128, EC, H], BF, False)  # blockdiag sel

    attn_o = nc.declare_dram_parameter("attn", [H, TL, T], BF, True)
    out_o = nc.declare_dram_parameter("out", [TL, E], F32, True)

    from contextlib import ExitStack

    with ExitStack() as top:
        tc = top.enter_context(tile.TileContext(nc))

        consts = top.enter_context(tc.tile_pool(name="consts", bufs=1))
        persist = top.enter_context(tc.tile_pool(name="persist", bufs=1))
        dscr = top.enter_context(tc.tile_pool(name="dscr", bufs=1, space="DRAM"))

        id_sb = consts.tile([128, 128], BF)
        nc.sync.dma_start(out=id_sb, in_=ident[:])
        idf_sb = consts.tile([128, 128], mybir.dt.float32r)
        nc.sync.dma_start(out=idf_sb, in_=identf[:])
        ones_sb = consts.tile([128, 1], BF)
        nc.sync.dma_start(out=ones_sb, in_=ones1[:])
        bd_sb = consts.tile([128, EC, H], BF)
        nc.sync.dma_start(out=bd_sb, in_=bd[:])

        wout_sb = consts.tile([128, EC, E], BF)
        for kc in range(EC):
            nc.sync.dma_start(
                out=wout_sb[:, kc, :], in_=woutT[kc * 128:(kc + 1) * 128, :]
            )

        # persistent products of the prep phase
        qhT_sb = persist.tile([128, EC, T], BF)      # normalized q^T (24KB/p)
        v_sb = persist.tile([128, TC, E], BF)        # v, natural layout
        if need_clip:
            eb_sb = persist.tile([128, ICN, T], BF)  # exp-bias factor
        else:
            logeb_sb = persist.tile(
                [128, ICN, T], mybir.dt.float32r
            )  # additive bias / c

        # ================= PREP =================
        with ExitStack() as prep:
            wpool = prep.enter_context(tc.tile_pool(name="wpool", bufs=1))
            wqk_sb = wpool.tile([128, EC, E], BF)
            wv_sb = wpool.tile([128, EC, E], BF)
            for kc in range(EC):
                nc.sync.dma_start(
                    out=wqk_sb[:, kc, :], in_=wqkT[kc * 128:(kc + 1) * 128, :]
                )
                nc.sync.dma_start(
                    out=wv_sb[:, kc, :], in_=wvT[kc * 128:(kc + 1) * 128, :]
                )

            bigH = prep.enter_context(tc.tile_pool(name="bigH", bufs=1))
            hT_sb = bigH.tile([128, EC, T], BF)

            # ---- phase 1: RMS norm (x^T -> h^T) ----
            with ExitStack() as s1ctx:
                bigX = s1ctx.enter_context(tc.tile_pool(name="bigX", bufs=1))
                mpool = s1ctx.enter_context(
                    tc.tile_pool(name="msum_ps", bufs=1, space="PSUM")
                )
                spool = s1ctx.enter_context(tc.tile_pool(name="rs_small", bufs=1))
                xT_sb = bigX.tile([128, EC, T], BF)
                sqx_sb = bigX.tile([128, EC, T], BF)
                for ec in range(EC):
                    nc.sync.dma_start(
                        out=xT_sb[:, ec, :], in_=xT[ec * 128:(ec + 1) * 128, :]
                    )
                    nc.scalar.activation(
                        sqx_sb[:, ec, :], xT_sb[:, ec, :], AF.Square
                    )
                msum_ps = mpool.tile([1, T], F32)
                for wc in range(JW):
                    for ec in range(EC):
                        nc.tensor.matmul(
                            msum_ps[:, wc * NW:(wc + 1) * NW],
                            ones_sb,
                            sqx_sb[:, ec, wc * NW:(wc + 1) * NW],
                            start=(ec == 0),
                            stop=(ec == EC - 1),
                        )
                # rs_x = 1/sqrt(mean + 1e-6)
                eps1 = spool.tile([1, 1], F32)
                nc.vector.memset(eps1, 1e-6)
                srt = spool.tile([1, T], F32)
                for wc in range(JW):
                    sl = slice(wc * NW, (wc + 1) * NW)
                    nc.scalar.activation(
                        srt[:, sl], msum_ps[:, sl], AF.Sqrt,
                        bias=eps1[:], scale=1.0 / E
                    )
                rsx = spool.tile([1, T], F32)
                nc.vector.reciprocal_approx_fast(rsx, srt)
                rsxb = spool.tile([1, T], BF)
                nc.vector.tensor_copy(rsxb, rsx)
                rsx_d = dscr.tile([1, T], BF)
                nc.sync.dma_start(out=rsx_d, in_=rsxb)
                rsx_e = spool.tile([128, T], BF)
                nc.sync.dma_start(out=rsx_e, in_=_bcast_part(rsx_d[:], 128))
                for ec in range(EC):
                    nc.vector.tensor_mul(
                        hT_sb[:, ec, :], xT_sb[:, ec, :], rsx_e
                    )

            # ---- phase 2: projections + q normalize + v ----
            with ExitStack() as s2ctx:
                mm_ps = s2ctx.enter_context(
                    tc.tile_pool(name="mm_ps", bufs=2, space="PSUM")
                )
                nrm_ps = s2ctx.enter_context(
                    tc.tile_pool(name="nrm_ps", bufs=1, space="PSUM")
                )
                tr = s2ctx.enter_context(tc.tile_pool(name="prep_tr", bufs=2))

                for ec in range(EC):
                    shT = tr.tile([128, T], BF, tag="shT")
                    sqs = tr.tile([128, T], BF, tag="sqs")
                    nrm2 = nrm_ps.tile([2, T], F32, tag="nrm")
                    for wc in range(JW):
                        ps = mm_ps.tile([128, NW], F32, tag="mm")
                        for kc in range(EC):
                            nc.tensor.matmul(
                                ps,
                                wqk_sb[:, kc, ec * 128:(ec + 1) * 128],
                                hT_sb[:, kc, wc * NW:(wc + 1) * NW],
                                start=(kc == 0),
                                stop=(kc == EC - 1),
                            )
                        nc.vector.tensor_copy(shT[:, wc * NW:(wc + 1) * NW], ps)
                        nc.scalar.activation(
                            sqs[:, wc * NW:(wc + 1) * NW], ps, AF.Square
                        )
                        nc.tensor.matmul(
                            nrm2[:, wc * NW:(wc + 1) * NW],
                            bd_sb[:, ec, 2 * ec:2 * ec + 2],
                            sqs[:, wc * NW:(wc + 1) * NW],
                            start=True,
                            stop=True,
                        )
                    # rsq = 1/sqrt(nrm2)  [2, T]
                    srq = tr.tile([2, T], F32, tag="srq")
                    for wc in range(JW):
                        sl = slice(wc * NW, (wc + 1) * NW)
                        nc.scalar.activation(srq[:, sl], nrm2[:, sl], AF.Sqrt)
                    rsq = tr.tile([2, T], F32, tag="rsq")
                    nc.vector.reciprocal_approx_fast(rsq, srq)
                    rsqb = tr.tile([2, T], BF, tag="rsqb")
                    nc.vector.tensor_copy(rsqb, rsq)
                    rsq_d = dscr.tile([2, T], BF, tag="rsq_d")
                    nc.sync.dma_start(out=rsq_d, in_=rsqb)
                    rsq_e = tr.tile([128, T], BF, tag="rsq_e")
                    nc.sync.dma_start(
                        out=rsq_e[0:64, :], in_=_bcast_part(rsq_d[0:1, :], 64)
                    )
                    nc.sync.dma_start(
                        out=rsq_e[64:128, :], in_=_bcast_part(rsq_d[1:2, :], 64)
                    )
                    nc.vector.tensor_mul(qhT_sb[:, ec, :], shT, rsq_e)

                # ---- v projection (natural layout) ----
                for tcl in range(TC):
                    ps = mm_ps.tile([128, E], F32, tag="mm")
                    for n0 in range(0, E, 512):
                        n1 = min(n0 + 512, E)
                        for kc in range(EC):
                            nc.tensor.matmul(
                                ps[:, n0:n1],
                                hT_sb[:, kc, tcl * 128:(tcl + 1) * 128],
                                wv_sb[:, kc, n0:n1],
                                start=(kc == 0),
                                stop=(kc == EC - 1),
                            )
                    for n0 in range(0, E, 512):
                        n1 = min(n0 + 512, E)
                        nc.scalar.copy(v_sb[:, tcl, n0:n1], ps[:, n0:n1])

            # ---- phase 3: emotion bias stats + exp-bias factor ----
            with ExitStack() as s3ctx:
                sm = s3ctx.enter_context(tc.tile_pool(name="emo_small", bufs=1))
                ebtr = s3ctx.enter_context(tc.tile_pool(name="eb_tr", bufs=2))
                TC128 = T // 128
                e2d = sm.tile([128, TC128], F32)
                rk = sm.tile([128, TC128], F32)
                rk2 = sm.tile([128, TC128], F32)
                nc.sync.dma_start(out=e2d, in_=efull[:])
                nc.sync.dma_start(out=rk, in_=rowK[:])
                nc.sync.dma_start(out=rk2, in_=rowK2[:])
                t1 = sm.tile([128, TC128], F32)
                nc.vector.tensor_mul(t1, e2d, rk)
                s_eK = sm.tile([1, 1], F32)
                nc.gpsimd.tensor_reduce(
                    s_eK, t1, axis=mybir.AxisListType.XYZWC, op=OP.add
                )
                ee = sm.tile([128, TC128], F32)
                nc.vector.tensor_mul(ee, e2d, e2d)
                t2 = sm.tile([128, TC128], F32)
                nc.vector.tensor_mul(t2, ee, rk2)
                s_e2K2 = sm.tile([1, 1], F32)
                nc.gpsimd.tensor_reduce(
                    s_e2K2, t2, axis=mybir.AxisListType.XYZWC, op=OP.add
                )
                # var = (s_e2K2 - s_eK^2/T^2) / (T^2-1); std = max(sqrt,1e-6)
                u = sm.tile([1, 1], F32)
                nc.vector.tensor_mul(u, s_eK, s_eK)
                var = sm.tile([1, 1], F32)
                nc.vector.scalar_tensor_tensor(
                    var, u, -1.0 / (T * T), s_e2K2, OP.mult, OP.add
                )
                std = sm.tile([1, 1], F32)
                nc.scalar.activation(std, var, AF.Sqrt, scale=1.0 / (T * T - 1))
                nc.vector.tensor_scalar_max(std, std, 1e-6)
                istd = sm.tile([1, 1], F32)
                nc.vector.reciprocal(istd, std)
                s1 = sm.tile([1, 1], F32)
                gam_eff = GAM if need_clip else GAM / c_scale
                nc.vector.tensor_scalar_mul(s1, istd, gam_eff)
                s1_d = dscr.tile([1, 1], F32)
                nc.sync.dma_start(out=s1_d, in_=s1)
                s1b = sm.tile([128, 1], F32)
                nc.sync.dma_start(out=s1b, in_=_bcast_part(s1_d[:], 128))

                # ---- exp-bias factor per local row chunk ----
                elc = sm.tile([128, ICN], F32)
                nc.sync.dma_start(
                    out=elc,
                    in_=eloc[:].rearrange("(c p) one -> p (c one)", p=128),
                )
                for icn in range(ICN):
                    kt = ebtr.tile([128, T], F32, tag="kt")
                    nc.sync.dma_start(
                        out=kt, in_=Kmat[icn * 128:(icn + 1) * 128, :]
                    )
                    s1e = sm.tile([128, 1], F32, tag="s1e")
                    nc.vector.tensor_mul(s1e, elc[:, icn:icn + 1], s1b)
                    if need_clip:
                        eb1 = ebtr.tile([128, T], BF, tag="eb1")
                        nc.scalar.activation(eb1, kt, AF.Exp, scale=s1e)
                        gt = ebtr.tile([128, T], BF, tag="gt")
                        nc.sync.dma_start(
                            out=gt, in_=expG[icn * 128:(icn + 1) * 128, :]
                        )
                        nc.vector.tensor_mul(eb_sb[:, icn, :], eb1, gt)
                    else:
                        gt = ebtr.tile([128, T], F32, tag="gtf")
                        nc.sync.dma_start(
                            out=gt, in_=logGc[icn * 128:(icn + 1) * 128, :]
                        )
                        tmp = ebtr.tile([128, T], F32, tag="ktmp")
                        nc.vector.tensor_scalar_mul(tmp, kt, s1e[:])
                        nc.vector.tensor_add(logeb_sb[:, icn, :], tmp, gt)

        # ================= HEAD LOOP =================
        with ExitStack() as hl:
            ps_s = hl.enter_context(tc.tile_pool(name="ps_s", bufs=2, space="PSUM"))
            ps_t = hl.enter_context(tc.tile_pool(name="ps_t", bufs=2, space="PSUM"))
            ps_av = hl.enter_context(
                tc.tile_pool(name="ps_av", bufs=2, space="PSUM")
            )
            hp = hl.enter_context(tc.tile_pool(name="hp", bufs=3))
            pTp = hl.enter_context(tc.tile_pool(name="pTp", bufs=2))
            mrg = hl.enter_context(tc.tile_pool(name="mrg", bufs=1))

            mergedT = mrg.tile([128, EC, TL], BF)

            for h in range(H):
                ec, half = h // 2, h % 2
                r0 = half * 64
                pT = pTp.tile([128, JB, TL], BF, tag="pT")
                for icn in range(ICN):
                    p_t = hp.tile([128, T], BF, tag="p")
                    rsum = hp.tile([128, JW], F32, tag="rsum")
                    for wc in range(JW):
                        ps = ps_s.tile([128, NW], F32, tag="s")
                        sl = slice(wc * NW, (wc + 1) * NW)
                        if need_clip:
                            nc.tensor.matmul(
                                ps,
                                qhT_sb[r0:r0 + 64, ec,
                                       icn * 128:(icn + 1) * 128],
                                qhT_sb[r0:r0 + 64, ec, sl],
                                start=True,
                                stop=True,
                            )
                            cl = hp.tile([128, NW], F32, tag="cl")
                            nc.vector.tensor_scalar(
                                cl, ps, c_scale, -c_scale, OP.mult, OP.add
                            )
                            nc.vector.tensor_scalar(
                                cl, cl, 12.0, -12.0, OP.min, OP.max
                            )
                            nc.scalar.activation(
                                p_t[:, sl], cl, AF.Exp,
                                accum_out=rsum[:, wc:wc + 1],
                            )
                        else:
                            nc.tensor.matmul(
                                ps,
                                qhT_sb[r0:r0 + 64, ec,
                                       icn * 128:(icn + 1) * 128],
                                qhT_sb[r0:r0 + 64, ec, sl],
                                start=True,
                                stop=False,
                                skip_group_check=True,
                            )
                            nc.tensor.matmul(
                                ps,
                                idf_sb[:],
                                logeb_sb[:, icn, sl],
                                start=False,
                                stop=True,
                                skip_group_check=True,
                            )
                            nc.scalar.activation(
                                p_t[:, sl], ps, AF.Exp, scale=c_scale,
                                accum_out=rsum[:, wc:wc + 1],
                            )
                    if need_clip:
                        nc.vector.tensor_mul(p_t, p_t, eb_sb[:, icn, :])
                        rtot = hp.tile([128, 1], F32, tag="rtot")
                        nc.vector.reduce_sum(
                            rtot, p_t, axis=mybir.AxisListType.X
                        )
                    else:
                        rtot = hp.tile([128, 1], F32, tag="rtot")
                        nc.vector.reduce_sum(
                            rtot, rsum, axis=mybir.AxisListType.X
                        )
                    recip = hp.tile([128, 1], F32, tag="recip")
                    nc.vector.reciprocal_approx_fast(recip, rtot)
                    pn = hp.tile([128, T], BF, tag="pn")
                    nc.vector.tensor_scalar_mul(pn, p_t, recip)
                    nc.sync.dma_start(
                        out=attn_o[h, icn * 128:(icn + 1) * 128, :], in_=pn
                    )
                    # transpose pn into pT
                    for jg in range(JB // 8):
                        pst = ps_t.tile([128, 8, 128], BF, tag="t")
                        for jb8 in range(8):
                            jb = jg * 8 + jb8
                            nc.tensor.transpose(
                                pst[:, jb8, :],
                                pn[:, jb * 128:(jb + 1) * 128],
                                id_sb,
                            )
                        nc.vector.tensor_copy(
                            pT[:, jg * 8:(jg + 1) * 8,
                               icn * 128:(icn + 1) * 128],
                            pst,
                        )
                # attn @ v (transposed output: [64, TL])
                av = ps_av.tile([64, TL], F32, tag="av")
                for jc in range(JB):
                    nc.tensor.matmul(
                        av,
                        v_sb[:, jc, h * 64:(h + 1) * 64],
                        pT[:, jc, :],
                        start=(jc == 0),
                        stop=(jc == JB - 1),
                    )
                nc.scalar.copy(mergedT[r0:r0 + 64, ec, :], av)

            # ---- out projection ----
            op_sb = hl.enter_context(tc.tile_pool(name="op_sb", bufs=2))
            for icn in range(ICN):
                ps = ps_s.tile([128, E], F32, tag="s")
                for n0 in range(0, E, 512):
                    n1 = min(n0 + 512, E)
                    for kc in range(EC):
                        nc.tensor.matmul(
                            ps[:, n0:n1],
                            mergedT[:, kc, icn * 128:(icn + 1) * 128],
                            wout_sb[:, kc, n0:n1],
                            start=(kc == 0),
                            stop=(kc == EC - 1),
                        )
                ot = op_sb.tile([128, E], F32, tag="ot")
                for n0 in range(0, E, 512):
                    n1 = min(n0 + 512, E)
                    nc.vector.tensor_copy(ot[:, n0:n1], ps[:, n0:n1])
                nc.sync.dma_start(
                    out=out_o[icn * 128:(icn + 1) * 128, :], in_=ot
                )

    nc.finalize()
    return nc


def _sigmoid(z):
    return 1.0 / (1.0 + np.exp(-z))


def make_host_inputs(x, emotion, w_qk, w_v, w_out, rms_weight, sigma, beta_local,
                     beta_global, T):
    """Build the per-core in_maps (list of dicts) plus compile-time scalars."""
    TL = T // NCORES
    x2 = np.asarray(x, np.float32).reshape(T, E)
    e = np.asarray(emotion, np.float32).reshape(T)

    sig = max(float(sigma), SIGMA_MIN)
    c = 2.0 / (sig * sig)
    need_clip = (4.0 / (sig * sig)) > 12.0

    rmsw = np.asarray(rms_weight, np.float32).reshape(E)
    wqkT = (np.asarray(w_qk, np.float32) * rmsw[None, :]).T.astype(BF16)
    wvT = (np.asarray(w_v, np.float32) * rmsw[None, :]).T.astype(BF16)
    woutT = np.asarray(w_out, np.float32).T.astype(BF16)
    wqkT = np.ascontiguousarray(wqkT)
    wvT = np.ascontiguousarray(wvT)
    woutT = np.ascontiguousarray(woutT)

    idx = np.arange(T)
    adiff = np.abs(idx[None, :] - idx[:, None]).astype(np.float32)
    local = (adiff <= WIN).astype(np.float32)
    strided = np.broadcast_to(
        ((idx % STRIDE) == 0).astype(np.float32)[None, :], (T, T)
    )
    gate = _sigmoid(beta_local) * local + _sigmoid(beta_global) * strided
    expG_full = np.maximum(gate, 1e-4).astype(np.float32)
    logGc_full = (np.log(expG_full) / c).astype(np.float32)
    Kfull = np.exp(-adiff / max(LAM, 1e-6)).astype(np.float32)
    rowK = Kfull.sum(1).astype(np.float32)
    rowK2 = (Kfull * Kfull).sum(1).astype(np.float32)

    xT_full = np.ascontiguousarray(x2.T).astype(BF16)  # [E, T]
    ident = np.eye(128, dtype=np.float32).astype(BF16)
    identf = np.eye(128, dtype=np.float32)
    ones1 = np.ones((128, 1), np.float32).astype(BF16)
    EC = E // 128
    bd = np.zeros((128, EC, H), np.float32)
    for ec in range(EC):
        bd[0:64, ec, 2 * ec] = 1.0
        bd[64:128, ec, 2 * ec + 1] = 1.0
    bd = bd.astype(BF16)

    e2d = np.ascontiguousarray(e.reshape(128, T // 128))
    rowK2d = np.ascontiguousarray(rowK.reshape(128, T // 128))
    rowK22d = np.ascontiguousarray(rowK2.reshape(128, T // 128))

    in_maps = []
    for r in range(NCORES):
        rows = slice(r * TL, (r + 1) * TL)
        in_maps.append(
            dict(
                xT=np.ascontiguousarray(np.roll(xT_full, -r * TL, axis=1)),
                wqkT=wqkT,
                wvT=wvT,
                woutT=woutT,
                expG=np.ascontiguousarray(
                    np.roll(expG_full[rows], -r * TL, axis=1)
                ).astype(BF16),
                logGc=np.ascontiguousarray(
                    np.roll(logGc_full[rows], -r * TL, axis=1)
                ),
                Kmat=np.ascontiguousarray(np.roll(Kfull[rows], -r * TL, axis=1)),
                eloc=np.ascontiguousarray(e[rows].reshape(TL, 1)),
                efull=e2d,
                rowK=rowK2d,
                rowK2=rowK22d,
                ident=ident,
                identf=identf,
                ones1=ones1,
                bd=bd,
            )
        )
    return in_maps, c, need_clip


def assemble_outputs(results, T):
    TL = T // NCORES
    out = np.zeros((1, T, E), np.float32)
    attn = np.zeros((1, H, T, T), np.float32)
    for r in range(NCORES):
        res = results[r]
        out[0, r * TL:(r + 1) * TL] = np.asarray(res["out"], np.float32)
        a = np.asarray(res["attn"], np.float32)  # [H, TL, T] rotated
        attn[0, :, r * TL:(r + 1) * TL, :] = np.roll(a, r * TL, axis=2)
    return out, attn


def kernel(x, emotion_activation, w_qk, w_v, w_out, rms_weight, sigma,
           beta_local, beta_global):
    global LAST_EXEC_NS
    x = np.asarray(x)
    B, T, _ = x.shape
    assert B == 1

    in_maps, c, need_clip = make_host_inputs(
        x, emotion_activation, w_qk, w_v, w_out, rms_weight, sigma,
        beta_local, beta_global, T
    )
    nc = build_bass(T, c, need_clip)

    trace = os.environ.get("KERNEL_TRACE", "0") == "1"
    if trace:
        _install_ntff_hook()
        global LAST_TRACE_DIR
        import tempfile

        LAST_TRACE_DIR = tempfile.mkdtemp(prefix="kernel_trace_")
        res = run_bass_kernel_spmd(
            nc, in_maps, core_ids=list(range(NCORES)), trace=True,
            tmpdir=LAST_TRACE_DIR,
        )
    else:
        res = run_bass_kernel_spmd(
            nc, in_maps, core_ids=list(range(NCORES)), trace=False
        )
    LAST_EXEC_NS = res.exec_time_ns
    out, attn = assemble_outputs(res.results, T)
    return out, attn


# revision 23
# speedup vs baseline: 1.3398x; 1.0084x over previous
"""Trainium2 Bass kernel for nn_AssocKernelAdapter (sparse_attention).

Strategy (8 NeuronCores, SPMD, no collectives):
  - Each core redundantly computes the prep (RMSNorm + qk/v projections +
    L2-normalize) for the full sequence in TRANSPOSED space (feature-major),
    then computes attention scores / softmax / attn@v / out-proj for its own
    T/8 = 256 query rows only (all 12 heads).
  - All cores run the same instruction graph; per-core behavior comes from a
    host-side cyclic rotation of the inputs: core r receives x^T (and the
    bias constants) rolled by -r*TL along the sequence axis so that its own
    query rows always occupy columns [0, TL). The host un-rotates the attn
    output when assembling.
  - Score math: softmax(clip(-dist/sig^2) + log(gate) + emo) is rewritten as
      softmax_j( c*sim[i,j] + log(gate[i,j]) + s1*e[i]*K[i,j] + const(i) )
    with c = 2/sig^2, K[i,j] = exp(-|i-j|/lam), s1 = GAM/std.  Row-constant
    terms cancel in softmax, so on device:
      p = exp(c*sim) * EB,   EB = max(gate,1e-4) * exp(s1*e_i*K)   (per core,
    shared across heads), then normalize by the row sum.
  - bf16 matmuls/storage, f32 reductions.  attn is written to HBM in bf16 and
    cast to f32 on host.
"""

import os
import sys

import numpy as np

sys.path.insert(0, "/opt/trn_rl_repo")

import concourse.bass as bass
import concourse.mybir as mybir
from concourse import bacc
import concourse.tile as tile
from concourse.bass_utils import run_bass_kernel_spmd

try:
    import ml_dtypes

    BF16 = ml_dtypes.bfloat16
except ImportError:  # pragma: no cover
    BF16 = np.float32

E = 768
H = 12
D = 64
WIN = 128
STRIDE = 8
LAM = 8.0
GAM = 0.5
SIGMA_MIN = 1e-4
NCORES = 8

F32 = mybir.dt.float32
BF = mybir.dt.bfloat16
AF = mybir.ActivationFunctionType
OP = mybir.AluOpType

LAST_EXEC_NS = None
LAST_TRACE_DIR = None


def _install_ntff_hook():
    """Provide antenv.axon_hooks (missing in this container) via ctypes."""
    import contextlib
    import ctypes
    import types

    if "antenv.axon_hooks" in sys.modules:
        return
    so_path = "/opt/axon/libaxon_pjrt.so"
    try:
        lib = ctypes.CDLL(so_path)
        if not hasattr(lib, "axon_start_nrt_profile"):
            return
    except OSError:
        return
    lib.axon_start_nrt_profile.argtypes = [
        ctypes.POINTER(ctypes.c_int64),
        ctypes.c_size_t,
    ]
    lib.axon_start_nrt_profile.restype = ctypes.c_int64
    lib.axon_stop_nrt_profile.argtypes = [ctypes.c_char_p]
    lib.axon_stop_nrt_profile.restype = ctypes.c_int64

    @contextlib.contextmanager
    def _hook(output_dir, device_ids):
        import jax

        jax.devices()
        if device_ids:
            ids = (ctypes.c_int64 * len(device_ids))(*device_ids)
            rc = lib.axon_start_nrt_profile(ids, len(device_ids))
        else:
            rc = lib.axon_start_nrt_profile(None, 0)
        if rc != 0:
            raise RuntimeError(f"axon_start_nrt_profile rc={rc}")
        try:
            yield
        finally:
            n = lib.axon_stop_nrt_profile(str(output_dir).encode())
            print(f"ntff profile: {n} file(s) written to {output_dir}",
                  file=sys.stderr)

    mod = types.ModuleType("antenv.axon_hooks")
    mod.get_axon_ntff_profile_hook = lambda: _hook
    mod.set_axon_ntff_profile_hook = lambda h: None
    sys.modules["antenv.axon_hooks"] = mod


def _bcast_part(ap, n):
    """View a [1, ...] SBUF/DRAM AP broadcast to n partitions (step-0)."""
    return bass.AP(
        tensor=ap.tensor,
        offset=ap.offset,
        ap=[[0, n]] + [list(x) for x in ap.ap[1:]],
    )


def build_bass(T, c_scale, need_clip=False):
    """Build the per-core Bass graph (identical on all cores)."""
    TL = T // NCORES          # local query rows per core
    ICN = max(TL // 128, 1)   # i-chunks of 128 local rows
    assert TL % 128 == 0
    EC = E // 128             # feature chunks (6)
    NW = min(512, T)          # matmul N chunk (PSUM bank = 512 f32)
    JW = T // NW              # j chunks per row tile
    JB = T // 128             # 128-wide j blocks (for transpose / attn@v)
    TC = T // 128             # full-sequence row chunks (for v)

    nc = bacc.Bacc(trn_type="TRN2")

    # ---- DRAM parameters (host supplies exact layouts) ----
    xT = nc.declare_dram_parameter("xT", [E, T], BF, False)       # rotated
    wqkT = nc.declare_dram_parameter("wqkT", [E, E], BF, False)   # [k, e], rms folded
    wvT = nc.declare_dram_parameter("wvT", [E, E], BF, False)
    woutT = nc.declare_dram_parameter("woutT", [E, E], BF, False)
    expG = nc.declare_dram_parameter("expG", [TL, T], BF, False)  # local rows, rotated
    logGc = nc.declare_dram_parameter("logGc", [TL, T], F32, False)  # log(gate)/c
    Kmat = nc.declare_dram_parameter("Kmat", [TL, T], F32, False)
    eloc = nc.declare_dram_parameter("eloc", [TL, 1], F32, False)
    efull = nc.declare_dram_parameter("efull", [128, T // 128], F32, False)
    rowK = nc.declare_dram_parameter("rowK", [128, T // 128], F32, False)
    rowK2 = nc.declare_dram_parameter("rowK2", [128, T // 128], F32, False)
    ident = nc.declare_dram_parameter("ident", [128, 128], BF, False)
    identf = nc.declare_dram_parameter(
        "identf", [128, 128], mybir.dt.float32r, False
    )
    ones1 = nc.declare_dram_parameter("ones1", [128, 1], BF, False)
    bd = nc.declare_dram_parameter("bd", [128, EC, H], BF, False)  # blockdiag sel

    attn_o = nc.declare_dram_parameter("attn", [H, TL, T], BF, True)
    out_o = nc.declare_dram_parameter("out", [TL, E], F32, True)

    from contextlib import ExitStack

    with ExitStack() as top:
        tc = top.enter_context(tile.TileContext(nc))

        consts = top.enter_context(tc.tile_pool(name="consts", bufs=1))
        persist = top.enter_context(tc.tile_pool(name="persist", bufs=1))
        dscr = top.enter_context(tc.tile_pool(name="dscr", bufs=2, space="DRAM"))

        id_sb = consts.tile([128, 128], BF)
        nc.sync.dma_start(out=id_sb, in_=ident[:])
        idf_sb = consts.tile([128, 128], mybir.dt.float32r)
        nc.sync.dma_start(out=idf_sb, in_=identf[:])
        ones_sb = consts.tile([128, 1], BF)
        nc.sync.dma_start(out=ones_sb, in_=ones1[:])
        bd_sb = consts.tile([128, EC, H], BF)
        nc.sync.dma_start(out=bd_sb, in_=bd[:])

        wout_sb = consts.tile([128, EC, E], BF)
        for kc in range(EC):
            nc.sync.dma_start(
                out=wout_sb[:, kc, :], in_=woutT[kc * 128:(kc + 1) * 128, :]
            )

        # persistent products of the prep phase
        qhT_sb = persist.tile([128, EC, T], BF)      # normalized q^T (24KB/p)
        v_sb = persist.tile([128, TC, E], BF)        # v, natural layout
        if need_clip:
            eb_sb = persist.tile([128, ICN, T], BF)  # exp-bias factor
        else:
            logeb_sb = persist.tile(
                [128, ICN, T], mybir.dt.float32r
            )  # additive bias / c

        # ================= PREP =================
        with ExitStack() as prep:
            wpool = prep.enter_context(tc.tile_pool(name="wpool", bufs=1))
            wqk_sb = wpool.tile([128, EC, E], BF)
            wv_sb = wpool.tile([128, EC, E], BF)
            for kc in range(EC):
                nc.sync.dma_start(
                    out=wqk_sb[:, kc, :], in_=wqkT[kc * 128:(kc + 1) * 128, :]
                )
                nc.sync.dma_start(
                    out=wv_sb[:, kc, :], in_=wvT[kc * 128:(kc + 1) * 128, :]
                )

            bigH = prep.enter_context(tc.tile_pool(name="bigH", bufs=1))
            hT_sb = bigH.tile([128, EC, T], BF)

            # ---- phase 1: RMS norm (x^T -> h^T) ----
            with ExitStack() as s1ctx:
                bigX = s1ctx.enter_context(tc.tile_pool(name="bigX", bufs=1))
                mpool = s1ctx.enter_context(
                    tc.tile_pool(name="msum_ps", bufs=1, space="PSUM")
                )
                spool = s1ctx.enter_context(tc.tile_pool(name="rs_small", bufs=1))
                xT_sb = bigX.tile([128, EC, T], BF)
                sqx_sb = bigX.tile([128, EC, T], BF)
                for ec in range(EC):
                    nc.sync.dma_start(
                        out=xT_sb[:, ec, :], in_=xT[ec * 128:(ec + 1) * 128, :]
                    )
                    nc.scalar.activation(
                        sqx_sb[:, ec, :], xT_sb[:, ec, :], AF.Square
                    )
                msum_ps = mpool.tile([1, T], F32)
                for wc in range(JW):
                    for ec in range(EC):
                        nc.tensor.matmul(
                            msum_ps[:, wc * NW:(wc + 1) * NW],
                            ones_sb,
                            sqx_sb[:, ec, wc * NW:(wc + 1) * NW],
                            start=(ec == 0),
                            stop=(ec == EC - 1),
                        )
                # rs_x = 1/sqrt(mean + 1e-6)
                eps1 = spool.tile([1, 1], F32)
                nc.vector.memset(eps1, 1e-6)
                srt = spool.tile([1, T], F32)
                for wc in range(JW):
                    sl = slice(wc * NW, (wc + 1) * NW)
                    nc.scalar.activation(
                        srt[:, sl], msum_ps[:, sl], AF.Sqrt,
                        bias=eps1[:], scale=1.0 / E
                    )
                rsx = spool.tile([1, T], F32)
                nc.vector.reciprocal_approx_fast(rsx, srt)
                rsxb = spool.tile([1, T], BF)
                nc.vector.tensor_copy(rsxb, rsx)
                rsx_d = dscr.tile([1, T], BF)
                nc.sync.dma_start(out=rsx_d, in_=rsxb)
                rsx_e = spool.tile([128, T], BF)
                nc.sync.dma_start(out=rsx_e, in_=_bcast_part(rsx_d[:], 128))
                for ec in range(EC):
                    nc.vector.tensor_mul(
                        hT_sb[:, ec, :], xT_sb[:, ec, :], rsx_e
                    )

            # ---- phase 2: projections + q normalize + v ----
            with ExitStack() as s2ctx:
                mm_ps = s2ctx.enter_context(
                    tc.tile_pool(name="mm_ps", bufs=2, space="PSUM")
                )
                nrm_ps = s2ctx.enter_context(
                    tc.tile_pool(name="nrm_ps", bufs=1, space="PSUM")
                )
                tr = s2ctx.enter_context(tc.tile_pool(name="prep_tr", bufs=2))

                for ec in range(EC):
                    shT = tr.tile([128, T], BF, tag="shT")
                    sqs = tr.tile([128, T], BF, tag="sqs")
                    nrm2 = nrm_ps.tile([2, T], F32, tag="nrm")
                    for wc in range(JW):
                        ps = mm_ps.tile([128, NW], F32, tag="mm")
                        for kc in range(EC):
                            nc.tensor.matmul(
                                ps,
                                wqk_sb[:, kc, ec * 128:(ec + 1) * 128],
                                hT_sb[:, kc, wc * NW:(wc + 1) * NW],
                                start=(kc == 0),
                                stop=(kc == EC - 1),
                            )
                        nc.vector.tensor_copy(shT[:, wc * NW:(wc + 1) * NW], ps)
                        nc.scalar.activation(
                            sqs[:, wc * NW:(wc + 1) * NW], ps, AF.Square
                        )
                        nc.tensor.matmul(
                            nrm2[:, wc * NW:(wc + 1) * NW],
                            bd_sb[:, ec, 2 * ec:2 * ec + 2],
                            sqs[:, wc * NW:(wc + 1) * NW],
                            start=True,
                            stop=True,
                        )
                    # rsq = 1/sqrt(nrm2)  [2, T]
                    srq = tr.tile([2, T], F32, tag="srq")
                    for wc in range(JW):
                        sl = slice(wc * NW, (wc + 1) * NW)
                        nc.scalar.activation(srq[:, sl], nrm2[:, sl], AF.Sqrt)
                    rsq = tr.tile([2, T], F32, tag="rsq")
                    nc.vector.reciprocal_approx_fast(rsq, srq)
                    rsqb = tr.tile([2, T], BF, tag="rsqb")
                    nc.vector.tensor_copy(rsqb, rsq)
                    rsq_d = dscr.tile([2, T], BF, tag="rsq_d")
                    nc.sync.dma_start(out=rsq_d, in_=rsqb)
                    rsq_e = tr.tile([128, T], BF, tag="rsq_e")
                    nc.sync.dma_start(
                        out=rsq_e[0:64, :], in_=_bcast_part(rsq_d[0:1, :], 64)
                    )
                    nc.sync.dma_start(
                        out=rsq_e[64:128, :], in_=_bcast_part(rsq_d[1:2, :], 64)
                    )
                    nc.vector.tensor_mul(qhT_sb[:, ec, :], shT, rsq_e)

                # ---- v projection (natural layout) ----
                for tcl in range(TC):
                    ps = mm_ps.tile([128, E], F32, tag="mm")
                    for n0 in range(0, E, 512):
                        n1 = min(n0 + 512, E)
                        for kc in range(EC):
                            nc.tensor.matmul(
                                ps[:, n0:n1],
                                hT_sb[:, kc, tcl * 128:(tcl + 1) * 128],
                                wv_sb[:, kc, n0:n1],
                                start=(kc == 0),
                                stop=(kc == EC - 1),
                            )
                    for n0 in range(0, E, 512):
                        n1 = min(n0 + 512, E)
                        nc.scalar.copy(v_sb[:, tcl, n0:n1], ps[:, n0:n1])

            # ---- phase 3: emotion bias stats + exp-bias factor ----
            with ExitStack() as s3ctx:
                sm = s3ctx.enter_context(tc.tile_pool(name="emo_small", bufs=1))
                ebtr = s3ctx.enter_context(tc.tile_pool(name="eb_tr", bufs=2))
                TC128 = T // 128
                e2d = sm.tile([128, TC128], F32)
                rk = sm.tile([128, TC128], F32)
                rk2 = sm.tile([128, TC128], F32)
                nc.sync.dma_start(out=e2d, in_=efull[:])
                nc.sync.dma_start(out=rk, in_=rowK[:])
                nc.sync.dma_start(out=rk2, in_=rowK2[:])
                t1 = sm.tile([128, TC128], F32)
                nc.vector.tensor_mul(t1, e2d, rk)
                s_eK = sm.tile([1, 1], F32)
                nc.gpsimd.tensor_reduce(
                    s_eK, t1, axis=mybir.AxisListType.XYZWC, op=OP.add
                )
                ee = sm.tile([128, TC128], F32)
                nc.vector.tensor_mul(ee, e2d, e2d)
                t2 = sm.tile([128, TC128], F32)
                nc.vector.tensor_mul(t2, ee, rk2)
                s_e2K2 = sm.tile([1, 1], F32)
                nc.gpsimd.tensor_reduce(
                    s_e2K2, t2, axis=mybir.AxisListType.XYZWC, op=OP.add
                )
                # var = (s_e2K2 - s_eK^2/T^2) / (T^2-1); std = max(sqrt,1e-6)
                u = sm.tile([1, 1], F32)
                nc.vector.tensor_mul(u, s_eK, s_eK)
                var = sm.tile([1, 1], F32)
                nc.vector.scalar_tensor_tensor(
                    var, u, -1.0 / (T * T), s_e2K2, OP.mult, OP.add
                )
                std = sm.tile([1, 1], F32)
                nc.scalar.activation(std, var, AF.Sqrt, scale=1.0 / (T * T - 1))
                nc.vector.tensor_scalar_max(std, std, 1e-6)
                istd = sm.tile([1, 1], F32)
                nc.vector.reciprocal(istd, std)
                s1 = sm.tile([1, 1], F32)
                gam_eff = GAM if need_clip else GAM / c_scale
                nc.vector.tensor_scalar_mul(s1, istd, gam_eff)
                s1_d = dscr.tile([1, 1], F32)
                nc.sync.dma_start(out=s1_d, in_=s1)
                s1b = sm.tile([128, 1], F32)
                nc.sync.dma_start(out=s1b, in_=_bcast_part(s1_d[:], 128))

                # ---- exp-bias factor per local row chunk ----
                elc = sm.tile([128, ICN], F32)
                nc.sync.dma_start(
                    out=elc,
                    in_=eloc[:].rearrange("(c p) one -> p (c one)", p=128),
                )
                for icn in range(ICN):
                    kt = ebtr.tile([128, T], F32, tag="kt")
                    nc.sync.dma_start(
                        out=kt, in_=Kmat[icn * 128:(icn + 1) * 128, :]
                    )
                    s1e = sm.tile([128, 1], F32, tag="s1e")
                    nc.vector.tensor_mul(s1e, elc[:, icn:icn + 1], s1b)
                    if need_clip:
                        eb1 = ebtr.tile([128, T], BF, tag="eb1")
                        nc.scalar.activation(eb1, kt, AF.Exp, scale=s1e)
                        gt = ebtr.tile([128, T], BF, tag="gt")
                        nc.sync.dma_start(
                            out=gt, in_=expG[icn * 128:(icn + 1) * 128, :]
                        )
                        nc.vector.tensor_mul(eb_sb[:, icn, :], eb1, gt)
                    else:
                        gt = ebtr.tile([128, T], F32, tag="gtf")
                        nc.sync.dma_start(
                            out=gt, in_=logGc[icn * 128:(icn + 1) * 128, :]
                        )
                        tmp = ebtr.tile([128, T], F32, tag="ktmp")
                        nc.vector.tensor_scalar_mul(tmp, kt, s1e[:])
                        nc.vector.tensor_add(logeb_sb[:, icn, :], tmp, gt)

        # ================= HEAD LOOP =================
        NWW = min(1024, T)
        JWW = T // NWW
        with ExitStack() as hl:
            ps_s = hl.enter_context(tc.tile_pool(name="ps_s", bufs=2, space="PSUM"))
            ps_t = hl.enter_context(tc.tile_pool(name="ps_t", bufs=2, space="PSUM"))
            ps_av = hl.enter_context(
                tc.tile_pool(name="ps_av", bufs=2, space="PSUM")
            )
            hp = hl.enter_context(tc.tile_pool(name="hp", bufs=3))
            pTp = hl.enter_context(tc.tile_pool(name="pTp", bufs=2))
            mrg = hl.enter_context(tc.tile_pool(name="mrg", bufs=1))

            mergedT = mrg.tile([128, EC, TL], BF)

            for h in range(H):
                ec, half = h // 2, h % 2
                r0 = half * 64
                pT = pTp.tile([128, JB, TL], BF, tag="pT")
                for icn in range(ICN):
                    p_t = hp.tile([128, T], BF, tag="p")
                    rsum = hp.tile([128, JWW], F32, tag="rsum")
                    for wc in range(JWW):
                        ps = ps_s.tile([128, NWW], F32, tag="s")
                        sl = slice(wc * NWW, (wc + 1) * NWW)
                        if need_clip:
                            for sub in range(NWW // NW):
                                o = sub * NW
                                nc.tensor.matmul(
                                    ps[:, o:o + NW],
                                    qhT_sb[r0:r0 + 64, ec,
                                           icn * 128:(icn + 1) * 128],
                                    qhT_sb[r0:r0 + 64, ec,
                                           wc * NWW + o:wc * NWW + o + NW],
                                    start=True,
                                    stop=True,
                                )
                            cl = hp.tile([128, NWW], F32, tag="cl")
                            nc.vector.tensor_scalar(
                                cl, ps, c_scale, -c_scale, OP.mult, OP.add
                            )
                            nc.vector.tensor_scalar(
                                cl, cl, 12.0, -12.0, OP.min, OP.max
                            )
                            nc.scalar.activation(
                                p_t[:, sl], cl, AF.Exp,
                                accum_out=rsum[:, wc:wc + 1],
                            )
                        else:
                            for sub in range(NWW // NW):
                                o = sub * NW
                                j0 = wc * NWW + o
                                nc.tensor.matmul(
                                    ps[:, o:o + NW],
                                    qhT_sb[r0:r0 + 64, ec,
                                           icn * 128:(icn + 1) * 128],
                                    qhT_sb[r0:r0 + 64, ec, j0:j0 + NW],
                                    start=True,
                                    stop=False,
                                    skip_group_check=True,
                                )
                                nc.tensor.matmul(
                                    ps[:, o:o + NW],
                                    idf_sb[:],
                                    logeb_sb[:, icn, j0:j0 + NW],
                                    start=False,
                                    stop=True,
                                    skip_group_check=True,
                                )
                            nc.scalar.activation(
                                p_t[:, sl], ps, AF.Exp, scale=c_scale,
                                accum_out=rsum[:, wc:wc + 1],
                            )
                    if need_clip:
                        nc.vector.tensor_mul(p_t, p_t, eb_sb[:, icn, :])
                        rtot = hp.tile([128, 1], F32, tag="rtot")
                        nc.vector.reduce_sum(
                            rtot, p_t, axis=mybir.AxisListType.X
                        )
                    else:
                        rtot = hp.tile([128, 1], F32, tag="rtot")
                        nc.vector.reduce_sum(
                            rtot, rsum, axis=mybir.AxisListType.X
                        )
                    recip = hp.tile([128, 1], F32, tag="recip")
                    nc.vector.reciprocal_approx_fast(recip, rtot)
                    pn = hp.tile([128, T], BF, tag="pn")
                    nc.vector.tensor_scalar_mul(pn, p_t, recip)
                    nc.gpsimd.dma_start(
                        out=attn_o[h, icn * 128:(icn + 1) * 128, :], in_=pn
                    )
                    # transpose pn into pT
                    for jg in range(JB // 8):
                        pst = ps_t.tile([128, 8, 128], BF, tag="t")
                        for jb8 in range(8):
                            jb = jg * 8 + jb8
                            nc.tensor.transpose(
                                pst[:, jb8, :],
                                pn[:, jb * 128:(jb + 1) * 128],
                                id_sb,
                            )
                        nc.vector.tensor_copy(
                            pT[:, jg * 8:(jg + 1) * 8,
                               icn * 128:(icn + 1) * 128],
                            pst,
                        )
                # attn @ v (transposed output: [64, TL])
                av = ps_av.tile([64, TL], F32, tag="av")
                for jc in range(JB):
                    nc.tensor.matmul(
                        av,
                        v_sb[:, jc, h * 64:(h + 1) * 64],
                        pT[:, jc, :],
                        start=(jc == 0),
                        stop=(jc == JB - 1),
                    )
                nc.vector.tensor_copy(mergedT[r0:r0 + 64, ec, :], av)

            # ---- out projection ----
            op_sb = hl.enter_context(tc.tile_pool(name="op_sb", bufs=2))
            for icn in range(ICN):
                ps = ps_s.tile([128, E], F32, tag="s")
                for n0 in range(0, E, 512):
                    n1 = min(n0 + 512, E)
                    for kc in range(EC):
                        nc.tensor.matmul(
                            ps[:, n0:n1],
                            mergedT[:, kc, icn * 128:(icn + 1) * 128],
                            wout_sb[:, kc, n0:n1],
                            start=(kc == 0),
                            stop=(kc == EC - 1),
                        )
                ot = op_sb.tile([128, E], F32, tag="ot")
                for n0 in range(0, E, 512):
                    n1 = min(n0 + 512, E)
                    nc.vector.tensor_copy(ot[:, n0:n1], ps[:, n0:n1])
                nc.sync.dma_start(
                    out=out_o[icn * 128:(icn + 1) * 128, :], in_=ot
                )

    nc.finalize()
    return nc


def _sigmoid(z):
    return 1.0 / (1.0 + np.exp(-z))


def make_host_inputs(x, emotion, w_qk, w_v, w_out, rms_weight, sigma, beta_local,
                     beta_global, T):
    """Build the per-core in_maps (list of dicts) plus compile-time scalars."""
    TL = T // NCORES
    x2 = np.asarray(x, np.float32).reshape(T, E)
    e = np.asarray(emotion, np.float32).reshape(T)

    sig = max(float(sigma), SIGMA_MIN)
    c = 2.0 / (sig * sig)
    need_clip = (4.0 / (sig * sig)) > 12.0

    rmsw = np.asarray(rms_weight, np.float32).reshape(E)
    wqkT = (np.asarray(w_qk, np.float32) * rmsw[None, :]).T.astype(BF16)
    wvT = (np.asarray(w_v, np.float32) * rmsw[None, :]).T.astype(BF16)
    woutT = np.asarray(w_out, np.float32).T.astype(BF16)
    wqkT = np.ascontiguousarray(wqkT)
    wvT = np.ascontiguousarray(wvT)
    woutT = np.ascontiguousarray(woutT)

    idx = np.arange(T)
    adiff = np.abs(idx[None, :] - idx[:, None]).astype(np.float32)
    local = (adiff <= WIN).astype(np.float32)
    strided = np.broadcast_to(
        ((idx % STRIDE) == 0).astype(np.float32)[None, :], (T, T)
    )
    gate = _sigmoid(beta_local) * local + _sigmoid(beta_global) * strided
    expG_full = np.maximum(gate, 1e-4).astype(np.float32)
    logGc_full = (np.log(expG_full) / c).astype(np.float32)
    Kfull = np.exp(-adiff / max(LAM, 1e-6)).astype(np.float32)
    rowK = Kfull.sum(1).astype(np.float32)
    rowK2 = (Kfull * Kfull).sum(1).astype(np.float32)

    xT_full = np.ascontiguousarray(x2.T).astype(BF16)  # [E, T]
    ident = np.eye(128, dtype=np.float32).astype(BF16)
    identf = np.eye(128, dtype=np.float32)
    ones1 = np.ones((128, 1), np.float32).astype(BF16)
    EC = E // 128
    bd = np.zeros((128, EC, H), np.float32)
    for ec in range(EC):
        bd[0:64, ec, 2 * ec] = 1.0
        bd[64:128, ec, 2 * ec + 1] = 1.0
    bd = bd.astype(BF16)

    e2d = np.ascontiguousarray(e.reshape(128, T // 128))
    rowK2d = np.ascontiguousarray(rowK.reshape(128, T // 128))
    rowK22d = np.ascontiguousarray(rowK2.reshape(128, T // 128))

    in_maps = []
    for r in range(NCORES):
        rows = slice(r * TL, (r + 1) * TL)
        in_maps.append(
            dict(
                xT=np.ascontiguousarray(np.roll(xT_full, -r * TL, axis=1)),
                wqkT=wqkT,
                wvT=wvT,
                woutT=woutT,
                expG=np.ascontiguousarray(
                    np.roll(expG_full[rows], -r * TL, axis=1)
                ).astype(BF16),
                logGc=np.ascontiguousarray(
                    np.roll(logGc_full[rows], -r * TL, axis=1)
                ),
                Kmat=np.ascontiguousarray(np.roll(Kfull[rows], -r * TL, axis=1)),
                eloc=np.ascontiguousarray(e[rows].reshape(TL, 1)),
                efull=e2d,
                rowK=rowK2d,
                rowK2=rowK22d,
                ident=ident,
                identf=identf,
                ones1=ones1,
                bd=bd,
            )
        )
    return in_maps, c, need_clip


def assemble_outputs(results, T):
    TL = T // NCORES
    out = np.zeros((1, T, E), np.float32)
    attn = np.zeros((1, H, T, T), np.float32)
    for r in range(NCORES):
        res = results[r]
        out[0, r * TL:(r + 1) * TL] = np.asarray(res["out"], np.float32)
        a = np.asarray(res["attn"], np.float32)  # [H, TL, T] rotated
        attn[0, :, r * TL:(r + 1) * TL, :] = np.roll(a, r * TL, axis=2)
    return out, attn


def kernel(x, emotion_activation, w_qk, w_v, w_out, rms_weight, sigma,
           beta_local, beta_global):
    global LAST_EXEC_NS
    x = np.asarray(x)
    B, T, _ = x.shape
    assert B == 1

    in_maps, c, need_clip = make_host_inputs(
        x, emotion_activation, w_qk, w_v, w_out, rms_weight, sigma,
        beta_local, beta_global, T
    )
    nc = build_bass(T, c, need_clip)

    trace = os.environ.get("KERNEL_TRACE", "0") == "1"
    if trace:
        _install_ntff_hook()
        global LAST_TRACE_DIR
        import tempfile

        LAST_TRACE_DIR = tempfile.mkdtemp(prefix="kernel_trace_")
        res = run_bass_kernel_spmd(
            nc, in_maps, core_ids=list(range(NCORES)), trace=True,
            tmpdir=LAST_TRACE_DIR,
        )
    else:
        res = run_bass_kernel_spmd(
            nc, in_maps, core_ids=list(range(NCORES)), trace=False
        )
    LAST_EXEC_NS = res.exec_time_ns
    out, attn = assemble_outputs(res.results, T)
    return out, attn
